# revision 1
# baseline (speedup 1.0000x reference)
"""8-core SPMD kernel for nn_NBST_79766132621711 (gnn_message_passing).

Strategy (per sharding hint): pure data parallel over batch B=32 across the
8 NeuronCores (4 samples per core).  The only cross-core coupling is the
training-mode BatchNorm statistics, which are computed exactly via
cross-device mean (lax.pmean) inside the SPMD program.  The GRU carries only
per-sample state, so batch sharding is free.

Self-contained: shapes hardcoded; accepts FULL inputs, returns FULL output.
"""

import numpy as np
import jax
import jax.numpy as jnp
from functools import partial

B, T, N = 32, 168, 50
HID, EMB = 128, 8
IN_DIM, NODE_IN = 11, 16
EPS = 1e-5
NCORES = 8
BL = B // NCORES  # 4 samples per core


def _ln(x, w, b):
    m = x.mean(-1, keepdims=True)
    v = x.var(-1, keepdims=True)
    return (x - m) * jax.lax.rsqrt(v + EPS) * w + b


def _bn_dist(x, w, b, axes, axis_name):
    # training-mode batch norm with stats over the GLOBAL batch: local moments
    # + cross-device mean.  Shards are equal-sized so pmean(local mean) is the
    # exact global mean; var computed two-pass like jnp.var.
    m = jax.lax.pmean(x.mean(axes, keepdims=True), axis_name)
    d = x - m
    v = jax.lax.pmean((d * d).mean(axes, keepdims=True), axis_name)
    return d * jax.lax.rsqrt(v + EPS) * w + b


def _bn_local(x, w, b, axes):
    m = x.mean(axes, keepdims=True)
    v = x.var(axes, keepdims=True)
    return (x - m) * jax.lax.rsqrt(v + EPS) * w + b


def _mlp(x, W1, b1, W2, b2, lnw, lnb, act):
    h = act(x @ W1 + b1)
    h = act(h @ W2 + b2)
    return _ln(h, lnw, lnb)


def _gru_cell(x, h, Wih, Whh, bih, bhh):
    gi = x @ Wih.T + bih
    gh = h @ Whh.T + bhh
    ir, iz, ic = jnp.split(gi, 3, axis=-1)
    hr, hz, hc = jnp.split(gh, 3, axis=-1)
    r = jax.nn.sigmoid(ir + hr)
    z = jax.nn.sigmoid(iz + hz)
    n = jnp.tanh(ic + r * hc)
    return (1.0 - z) * n + z * h


def _forward(local_node, local_features, local_emb, station_nodes,
             station_features, station_emb, p, bn):
    bs = local_node.shape[0]
    seq = station_features.shape[1]
    n_st = station_features.shape[2]

    month = p['month_emb'][station_emb[..., 3] - 1]
    day = p['day_emb'][station_emb[..., 4]]
    hour = p['hour_emb'][station_emb[..., 5]]
    pm25 = p['pm25_emb'][station_emb[..., 0]]
    st_w = p['weather_emb'][station_emb[..., 1]]
    st_wd = p['wind_emb'][station_emb[..., 2]]
    lo_w = p['weather_emb'][local_emb[..., 0]]
    lo_wd = p['wind_emb'][local_emb[..., 1]]

    sig = jax.nn.sigmoid
    local_node_emb = _mlp(local_node, p['st_W1'], p['st_b1'], p['st_W2'],
                          p['st_b2'], p['st_lnw'], p['st_lnb'], sig)
    station_nodes_emb = _mlp(station_nodes, p['st_W1'], p['st_b1'], p['st_W2'],
                             p['st_b2'], p['st_lnw'], p['st_lnb'], sig)

    scale = 1.0 / jnp.sqrt(jnp.asarray(n_st, local_node.dtype))
    attn_s = jax.nn.softmax(
        (local_node_emb @ jnp.swapaxes(station_nodes_emb, -1, -2)) * scale, axis=-1)
    static_out = attn_s @ station_nodes_emb
    static_norm = bn(static_out, p['bn_s_w'], p['bn_s_b'], (0, 1))

    sf_emb = _mlp(jnp.concatenate([station_features, st_w, st_wd], -1),
                  p['dy_W1'], p['dy_b1'], p['dy_W2'], p['dy_b2'],
                  p['dy_lnw'], p['dy_lnb'], jnp.tanh)
    lf_emb = _mlp(jnp.concatenate([local_features, lo_w, lo_wd], -1),
                  p['dy_W1'], p['dy_b1'], p['dy_W2'], p['dy_b2'],
                  p['dy_lnw'], p['dy_lnb'], jnp.tanh)

    attn_d = jax.nn.softmax(
        (lf_emb @ jnp.swapaxes(sf_emb, -1, -2)) * scale, axis=-1)
    dyn_out = attn_d @ sf_emb
    dyn_norm = bn(dyn_out, p['bn_d_w'], p['bn_d_b'], (0, 1, 2))

    nodes_rep = jnp.broadcast_to(station_nodes_emb[:, None],
                                 (bs, seq, n_st, HID))
    stations_feat = _mlp(jnp.concatenate([nodes_rep, sf_emb, pm25], -1),
                         p['sf_W1'], p['sf_b1'], p['sf_W2'], p['sf_b2'],
                         p['sf_lnw'], p['sf_lnb'], jax.nn.relu)

    stat_att = jnp.einsum('bn,btnh->bth', attn_s[:, 0], stations_feat)
    dyn_att = jnp.einsum('btn,btnh->bth', attn_d[:, :, 0], stations_feat)
    static_rep = jnp.broadcast_to(static_norm, (bs, seq, HID))

    tf = jnp.concatenate([static_rep, dyn_norm[:, :, 0], stat_att, dyn_att,
                          month[:, :, 0], day[:, :, 0], hour[:, :, 0]], axis=-1)

    xs = jnp.swapaxes(tf, 0, 1)
    h0 = (jnp.zeros((bs, HID), tf.dtype), jnp.zeros((bs, HID), tf.dtype))

    def step(carry, x):
        h1, h2 = carry
        h1 = _gru_cell(x, h1, p['gru_Wih0'], p['gru_Whh0'], p['gru_bih0'],
                       p['gru_bhh0'])
        h2 = _gru_cell(h1, h2, p['gru_Wih1'], p['gru_Whh1'], p['gru_bih1'],
                       p['gru_bhh1'])
        return (h1, h2), h2

    _, ys = jax.lax.scan(step, h0, xs)
    outs = jnp.swapaxes(ys, 0, 1)
    return jnp.tanh(outs @ p['pred_W'] + p['pred_b'])


def _forward_spmd(local_node, local_features, local_emb, station_nodes,
                  station_features, station_emb, p):
    bn = partial(_bn_dist, axis_name='b')
    return _forward(local_node, local_features, local_emb, station_nodes,
                    station_features, station_emb, p, bn)


_PMAP_CACHE = {}


def _get_pmap():
    if 'f' not in _PMAP_CACHE:
        _PMAP_CACHE['f'] = jax.pmap(
            _forward_spmd, axis_name='b',
            in_axes=(0, 0, 0, 0, 0, 0, None))
    return _PMAP_CACHE['f']


def _shard(x):
    x = np.asarray(x)
    return x.reshape((NCORES, BL) + x.shape[1:])


def kernel(local_node, local_features, local_emb, station_nodes,
           station_features, station_emb, params):
    params = jax.tree_util.tree_map(np.asarray, params)
    try:
        f = _get_pmap()
        out = f(_shard(local_node), _shard(local_features), _shard(local_emb),
                _shard(station_nodes), _shard(station_features),
                _shard(station_emb), params)
        out = np.asarray(out)
        return out.reshape(B, T, 1).astype(np.float32)
    except Exception:
        # Fallback: single-device (CPU) execution of the identical math with
        # global batch-norm statistics.
        with jax.default_device(jax.devices('cpu')[0]):
            out = _forward(jnp.asarray(local_node), jnp.asarray(local_features),
                           jnp.asarray(local_emb), jnp.asarray(station_nodes),
                           jnp.asarray(station_features),
                           jnp.asarray(station_emb), params, _bn_local)
            return np.asarray(out).reshape(B, T, 1).astype(np.float32)
